# revision 19
# baseline (speedup 1.0000x reference)
"""Trainium2 Bass kernel for nn_CM_sampler (retrieval_knn).

Counts, for each of 10000 class-blocked representatives (10 classes x 1000),
the other-class representatives within euclidean distance 0.5 (gram trick:
d2 = sq_i + sq_j - 2*X@X.T, count d2 < 0.25), then selects per class the
budget//C lowest-count rows (host argsort, tiny).

Layout: classes are zero-padded 1000 -> 1024 rows so query tiles are
[128 x D] (full PE width, FWL weight loads) and key chunks are 512 wide
(exactly one fp32 PSUM bank). Padded keys carry a -1e9 augment (never
counted); padded queries carry a +1e9 threshold (all-zero mask rows), so
padding never perturbs real counts.

Sharding: each of the 8 cores gets one 128-row query tile from EACH class
(rows [cls*1024 + core*128, +128)), so the class-skip pattern is
core-invariant and one SPMD program serves all 8 cores. All 10240 keys are
streamed on every core in 20 chunks of 512 (chunk c has class c//2).

Symmetry: the distance matrix is symmetric, so only class pairs
(query class t) < (key class c//2) are computed. Each [128 x 512] block
yields BOTH the row contribution (fused DVE is_gt + free-axis accumulate ->
counts for this core's class-t queries vs higher classes) and the column
contribution (ones^T @ mask matmul -> counts for the chunk's keys vs this
core's class-t queries). Column contributions are summed across cores on
the host; all counts are exact small integers in fp32.

Device math per block:
  PSUM  = 2*Q @ K^T - sq_k[None, :]   (matmul passes + augmented exact rows)
  mask  = [PSUM > (sq_q - 0.25)]      (fused DVE is_gt; accum_out = row sums)
  colsum += ones^T @ mask             (PE, accumulated per chunk in PSUM)

Matmul modes (pass structure for a*b):
  f32    - plain fp32 matmuls (4 cyc/row on TensorE).
  f32r   - TF32-like single pass (1 cyc/row): the HW rounds both operands
           to ~11-bit mantissas (RNE); d2 err ~2e-6, fastest but loosest.
  f32r3  - hi/lo split (RNE-at-13, grid-aligned below the HW's ~12-bit
           rounding): h@h + h@l + l@h, 3 cyc/row, fp32-grade.
  bf16x3 - same 3-pass split in bf16 (11x11-bit products are exact in the
           fp32 PSUM): fp32-grade (~3e-8 d2 err). SHIPPED MODE.
The -sq_k augmented rows are always split hi/residual so they are exact
under the mode's rounding.

Measured on the 8-core axon TRN2 (vs CPU-jax reference): HW exec ~575 us,
counts exact on 9914/10000 rows (rest off by 1 from borderline fp ties;
the 0.5 threshold deliberately straddles the data), ids_selected 1992/2000
exact, aggregate rel err ~2e-4. fp32 full-matrix baseline was 1.46 ms.
"""

import numpy as np

C, P, D = 10, 1000, 1024
N = C * P  # 10000 real rows
PP = 1024  # class size padded to 1024 (M=128 tiles -> FWL; KC=512 banks)
NP = C * PP  # 10240 padded rows
N_CORES = 8
QT = 128  # query tile rows (8 tiles per padded class -> 1 per core)
NQT = C  # query tiles per core (one per class)
QPC = QT * NQT  # 1280 queries per core
KC = 512  # key chunk (matmul free dim, exactly one fp32 PSUM bank)
NKC = NP // KC  # 20 chunks; chunk c has class c//2
KS = D // 128  # 8 contraction slices
PAD_NEG = np.float32(-1.0e9)  # kaug value for padded keys: never counted
PAD_POS = np.float32(1.0e9)  # qthr value for padded queries: all-zero mask rows

MM_MODE = "bf16x3"

_PROG_CACHE = {}


def _rne(a, k):
    """Round fp32 array to (23-k) explicit mantissa bits, round-to-nearest-even."""
    u = a.view(np.uint32).astype(np.uint64)
    bias = ((u >> k) & 1) + np.uint64((1 << (k - 1)) - 1)
    r = (((u + bias) >> k) << k) & np.uint64(0xFFFFFFFF)
    return r.astype(np.uint32).view(np.float32)


def _mode_cfg(mm_mode):
    return {
        "f32": dict(split=False, n_aug=1),
        "f32r": dict(split=False, n_aug=2),
        "f32r3": dict(split=True, n_aug=2),
        "bf16x3": dict(split=True, n_aug=3),
    }[mm_mode]


def _build_program(mm_mode):
    import concourse.mybir as mybir
    from concourse import bacc
    from concourse.tile import TileContext

    f32 = mybir.dt.float32
    if mm_mode == "f32":
        mm_dt = f32
    elif mm_mode == "bf16x3":
        mm_dt = mybir.dt.bfloat16
    else:
        mm_dt = mybir.dt.float32r
    cfg = _mode_cfg(mm_mode)
    split, n_aug = cfg["split"], cfg["n_aug"]

    nc = bacc.Bacc("TRN2", target_bir_lowering=False, debug=False, num_devices=N_CORES)

    # qT holds 2*Q^T k-slices (2x folded into the stationary operand so PSUM
    # accumulates 2G directly); in split modes qT/kT are hi parts, qTl/kTl
    # the residuals.
    qT = nc.dram_tensor("qT", [128, KS, QPC], mm_dt, kind="ExternalInput").ap()
    kT = nc.dram_tensor("kT", [128, KS, NP], mm_dt, kind="ExternalInput").ap()
    if split:
        qTl = nc.dram_tensor("qTl", [128, KS, QPC], mm_dt, kind="ExternalInput").ap()
        kTl = nc.dram_tensor("kTl", [128, KS, NP], mm_dt, kind="ExternalInput").ap()
    kaug = nc.dram_tensor("kaug", [n_aug, NP], mm_dt, kind="ExternalInput").ap()
    onesw = nc.dram_tensor("onesw", [n_aug, QT], mm_dt, kind="ExternalInput").ap()
    onecol = nc.dram_tensor("onecol", [QT, 1], mm_dt, kind="ExternalInput").ap()
    qthr = nc.dram_tensor("qthr", [QT, NQT], f32, kind="ExternalInput").ap()
    cnt_out = nc.dram_tensor("cnt", [QT, NQT], f32, kind="ExternalOutput").ap()
    col_out = nc.dram_tensor("colcnt", [1, NP], f32, kind="ExternalOutput").ap()

    with TileContext(nc) as tc:
        with (
            tc.tile_pool(name="qpool", bufs=1) as qpool,
            tc.tile_pool(name="kpool", bufs=2 if split else 3) as kpool,
            tc.tile_pool(name="ktpool", bufs=3) as ktpool,
            tc.tile_pool(name="mpool", bufs=4) as mpool,
            tc.tile_pool(name="small", bufs=1) as spool,
            tc.tile_pool(name="psum", bufs=5, space="PSUM") as pspool,
            tc.tile_pool(name="psum2", bufs=2, space="PSUM") as ps2pool,
        ):
            # Interleave per-slice q-hi / first-chunk k-hi DMAs so the first
            # matmul is gated on one slice pair (~460 KB), not the full 3.7 MB;
            # lo residuals queue after.
            qt_all = qpool.tile([128, KS, QPC], mm_dt, tag="qhi")
            kt0 = ktpool.tile([128, KS, KC], mm_dt, tag="kt")
            for ks in range(KS):
                nc.sync.dma_start(out=qt_all[:, ks, :], in_=qT[:, ks, :])
                nc.sync.dma_start(
                    out=kt0[:, ks, :], in_=kT[:, ks, 2 * KC : 3 * KC]
                )
            ka0 = kpool.tile([n_aug, KC], mm_dt, tag="ka")
            nc.sync.dma_start(out=ka0[:], in_=kaug[:, 2 * KC : 3 * KC])
            qtl_all = None
            if split:
                qtl_all = qpool.tile([128, KS, QPC], mm_dt, tag="qlo")
                nc.sync.dma_start(out=qtl_all[:], in_=qTl[:])

            qthr_t = spool.tile([QT, NQT], f32, tag="qthr")
            nc.sync.dma_start(out=qthr_t[:], in_=qthr[:])
            ones_t = spool.tile([n_aug, QT], mm_dt, tag="ones")
            nc.sync.dma_start(out=ones_t[:], in_=onesw[:])
            onecol_t = spool.tile([QT, 1], mm_dt, tag="onecol")
            nc.sync.dma_start(out=onecol_t[:], in_=onecol[:])

            acc = spool.tile([QT, NQT * 18], f32, tag="acc")
            nc.vector.memset(acc[:], 0.0)
            colcnt = spool.tile([1, NP], f32, tag="colcnt")
            cnt_t = spool.tile([QT, NQT], f32, tag="cnt")

            col = [0] * NQT
            for c in range(2, NKC):
                ts_list = list(range(c // 2))  # query classes strictly below
                if c == 2:
                    kt, ka = kt0, ka0
                else:
                    kt = ktpool.tile([128, KS, KC], mm_dt, tag="kt")
                    nc.sync.dma_start(
                        out=kt[:], in_=kT[:, :, c * KC : (c + 1) * KC]
                    )
                    ka = kpool.tile([n_aug, KC], mm_dt, tag="ka")
                    nc.sync.dma_start(out=ka[:], in_=kaug[:, c * KC : (c + 1) * KC])
                if split:
                    ktl = kpool.tile([128, KS, KC], mm_dt, tag="ktl")
                    nc.sync.dma_start(
                        out=ktl[:], in_=kTl[:, :, c * KC : (c + 1) * KC]
                    )

                masks = []
                for t in ts_list:
                    ps = pspool.tile([QT, KC], f32)
                    for ks in range(KS):
                        nc.tensor.matmul(
                            ps[:],
                            qt_all[:, ks, t * QT : (t + 1) * QT],
                            kt[:, ks, :],
                            start=(ks == 0),
                            stop=False,
                        )
                    if split:
                        for ks in range(KS):
                            nc.tensor.matmul(
                                ps[:],
                                qt_all[:, ks, t * QT : (t + 1) * QT],
                                ktl[:, ks, :],
                                start=False,
                                stop=False,
                            )
                        for ks in range(KS):
                            nc.tensor.matmul(
                                ps[:],
                                qtl_all[:, ks, t * QT : (t + 1) * QT],
                                kt[:, ks, :],
                                start=False,
                                stop=False,
                            )
                    nc.tensor.matmul(ps[:], ones_t[:], ka[:], start=False, stop=True)
                    mask = mpool.tile([QT, KC], mm_dt, tag="mask")
                    nc.vector.tensor_scalar(
                        out=mask[:],
                        in0=ps[:],
                        scalar1=qthr_t[:, t : t + 1],
                        scalar2=None,
                        op0=mybir.AluOpType.is_gt,
                        op1=mybir.AluOpType.add,
                        accum_out=acc[:, t * 18 + col[t] : t * 18 + col[t] + 1],
                    )
                    col[t] += 1
                    masks.append(mask)

                # column sums for this chunk's keys: ones^T @ mask, accumulated
                # over the chunk's query tiles in one PSUM group.
                ps2 = ps2pool.tile([1, KC], f32)
                for i, mask in enumerate(masks):
                    nc.tensor.matmul(
                        ps2[:],
                        onecol_t[:],
                        mask[:],
                        start=(i == 0),
                        stop=(i == len(masks) - 1),
                    )
                nc.vector.tensor_copy(colcnt[:, c * KC : (c + 1) * KC], ps2[:])

            for t in range(NQT):
                nc.vector.tensor_reduce(
                    cnt_t[:, t : t + 1],
                    acc[:, t * 18 : (t + 1) * 18],
                    axis=mybir.AxisListType.X,
                    op=mybir.AluOpType.add,
                )
            nc.sync.dma_start(out=cnt_out[:], in_=cnt_t[:])
            nc.sync.dma_start(out=col_out[:], in_=colcnt[:])

    nc.compile()
    return nc


def _get_program(mm_mode):
    if mm_mode not in _PROG_CACHE:
        _PROG_CACHE[mm_mode] = _build_program(mm_mode)
    return _PROG_CACHE[mm_mode]


def _split_hi_lo(a, mm_mode):
    if mm_mode == "bf16x3":
        import ml_dtypes

        hi = a.astype(ml_dtypes.bfloat16)
        lo = (a - hi.astype(np.float32)).astype(ml_dtypes.bfloat16)
        return np.ascontiguousarray(hi), np.ascontiguousarray(lo)
    hi = _rne(a, 13)
    return np.ascontiguousarray(hi), np.ascontiguousarray(a - hi)


def _aug_rows(msq, mm_mode):
    """Split -sq into n_aug rows whose sum is exact under the mode's rounding."""
    if mm_mode == "f32":
        return msq.reshape(1, -1).copy()
    if mm_mode == "bf16x3":
        import ml_dtypes

        h = msq.astype(ml_dtypes.bfloat16)
        r1 = msq - h.astype(np.float32)
        mid = r1.astype(ml_dtypes.bfloat16)
        lo = (r1 - mid.astype(np.float32)).astype(ml_dtypes.bfloat16)
        return np.ascontiguousarray(
            np.stack(
                [h.astype(np.float32), mid.astype(np.float32), lo.astype(np.float32)],
                axis=0,
            )
        )
    hi = _rne(msq, 13)
    return np.ascontiguousarray(np.stack([hi, msq - hi], axis=0))


def _np_dt(mm_mode):
    if mm_mode == "bf16x3":
        import ml_dtypes

        return ml_dtypes.bfloat16
    return np.float32


def _prepare_inputs(X, sq, mm_mode):
    """Build per-core in_maps from real X [N, D] f32 and sq [N] f32."""
    split = _mode_cfg(mm_mode)["split"]
    dt = _np_dt(mm_mode)

    # pad each class block from P=1000 to PP=1024 rows of zeros
    Xp = np.zeros((NP, D), dtype=np.float32)
    pad_mask = np.ones(NP, dtype=bool)  # True = padded row
    msq_p = np.full(NP, PAD_NEG, dtype=np.float32)  # -sq, PAD_NEG for pads
    thr_p = np.full(NP, PAD_POS, dtype=np.float32)  # sq-0.25, PAD_POS for pads
    for cls in range(C):
        Xp[cls * PP : cls * PP + P] = X[cls * P : (cls + 1) * P]
        pad_mask[cls * PP : cls * PP + P] = False
        msq_p[cls * PP : cls * PP + P] = -sq[cls * P : (cls + 1) * P]
        thr_p[cls * PP : cls * PP + P] = sq[cls * P : (cls + 1) * P] - np.float32(
            0.25
        )

    kT_full = np.ascontiguousarray(Xp.T.reshape(KS, 128, NP).transpose(1, 0, 2))
    if split:
        kT_hi, kT_lo = _split_hi_lo(kT_full, mm_mode)
    else:
        kT_hi = np.ascontiguousarray(kT_full.astype(dt))
        kT_lo = None

    kaug_full = np.ascontiguousarray(_aug_rows(msq_p, mm_mode).astype(dt))
    n_aug = kaug_full.shape[0]

    in_maps = []
    for core in range(N_CORES):
        rows = np.concatenate(
            [
                np.arange(cls * PP + core * QT, cls * PP + (core + 1) * QT)
                for cls in range(C)
            ]
        )
        Q2 = 2.0 * Xp[rows]  # exact scaling
        qT_c = np.ascontiguousarray(Q2.T.reshape(KS, 128, QPC).transpose(1, 0, 2))
        qthr_c = np.ascontiguousarray(thr_p[rows].reshape(NQT, QT).T)
        m = {
            "kT": kT_hi,
            "kaug": kaug_full,
            "qthr": qthr_c,
            "onesw": np.ones((n_aug, QT), dtype=dt),
            "onecol": np.ones((QT, 1), dtype=dt),
        }
        if split:
            m["qT"], m["qTl"] = _split_hi_lo(qT_c, mm_mode)
            m["kTl"] = kT_lo
        else:
            m["qT"] = np.ascontiguousarray(qT_c.astype(dt))
        in_maps.append(m)
    return in_maps


def _counts_from_results(results):
    counts_p = np.zeros(NP, dtype=np.int64)
    for core in range(N_CORES):  # row contributions (each row owned by 1 core)
        out = results[core]["cnt"]  # [QT, NQT] f32
        for cls in range(C):
            counts_p[cls * PP + core * QT : cls * PP + (core + 1) * QT] = out[
                :, cls
            ].astype(np.int64)
    for core in range(N_CORES):  # column contributions (summed over cores)
        cc = results[core]["colcnt"].reshape(NP)
        counts_p[PP:] += cc[PP:].astype(np.int64)  # class-0 keys have none
    # strip padding: real rows are the first P of each PP block
    return counts_p.reshape(C, PP)[:, :P].reshape(-1)


def kernel(feats, ids_per_cls, budget, _bench=None):
    from concourse.bass_utils import run_bass_kernel_spmd

    feats = np.asarray(feats, dtype=np.float32)
    ids_per_cls = np.asarray(ids_per_cls)
    budget_i = int(np.asarray(budget))

    ids_flat = ids_per_cls.reshape(-1).astype(np.int64)
    X = np.ascontiguousarray(feats[ids_flat])  # [N, D] class-blocked
    sq = (X.astype(np.float64) ** 2).sum(axis=1).astype(np.float32)

    nc = _get_program(MM_MODE)
    in_maps = _prepare_inputs(X, sq, MM_MODE)
    kw = dict(_bench) if _bench else {}
    res = run_bass_kernel_spmd(nc, in_maps, core_ids=list(range(N_CORES)), **kw)
    counts = _counts_from_results(res.results)

    counts = counts.reshape(C, P)
    per_cls_budget = budget_i // C
    order = np.argsort(counts, axis=-1, kind="stable")
    sel = order[:, :per_cls_budget]
    ids_selected = np.take_along_axis(
        ids_per_cls.reshape(C, P), sel, axis=1
    ).reshape(-1)

    cnt_dt = np.int64 if ids_per_cls.dtype == np.int64 else np.int32
    counts_out = counts.astype(cnt_dt)
    if _bench is not None:
        return (ids_selected, counts_out), res
    return ids_selected, counts_out


# revision 20
# speedup vs baseline: 1.0009x; 1.0009x over previous
"""Trainium2 Bass kernel for nn_CM_sampler (retrieval_knn).

Counts, for each of 10000 class-blocked representatives (10 classes x 1000),
the other-class representatives within euclidean distance 0.5 (gram trick:
d2 = sq_i + sq_j - 2*X@X.T, count d2 < 0.25), then selects per class the
budget//C lowest-count rows (host argsort, tiny).

Layout: classes are zero-padded 1000 -> 1024 rows so query tiles are
[128 x D] (full PE width, FWL weight loads) and key chunks are 512 wide
(exactly one fp32 PSUM bank). Padded keys carry a -1e9 augment (never
counted); padded queries carry a +1e9 threshold (all-zero mask rows), so
padding never perturbs real counts.

Sharding: each of the 8 cores gets one 128-row query tile from EACH class
(rows [cls*1024 + core*128, +128)), so the class-skip pattern is
core-invariant and one SPMD program serves all 8 cores. All 10240 keys are
streamed on every core in 20 chunks of 512 (chunk c has class c//2).

Symmetry: the distance matrix is symmetric, so only class pairs
(query class t) < (key class c//2) are computed. Each [128 x 512] block
yields BOTH the row contribution (fused DVE is_gt + free-axis accumulate ->
counts for this core's class-t queries vs higher classes) and the column
contribution (ones^T @ mask matmul -> counts for the chunk's keys vs this
core's class-t queries). Column contributions are summed across cores on
the host; all counts are exact small integers in fp32.

Device math per block:
  PSUM  = 2*Q @ K^T - sq_k[None, :]   (matmul passes + augmented exact rows)
  mask  = [PSUM > (sq_q - 0.25)]      (fused DVE is_gt; accum_out = row sums)
  colsum += ones^T @ mask             (PE, accumulated per chunk in PSUM)

Matmul modes (pass structure for a*b):
  f32    - plain fp32 matmuls (4 cyc/row on TensorE).
  f32r   - TF32-like single pass (1 cyc/row): the HW rounds both operands
           to ~11-bit mantissas (RNE); d2 err ~2e-6, fastest but loosest.
  f32r3  - hi/lo split (RNE-at-13, grid-aligned below the HW's ~12-bit
           rounding): h@h + h@l + l@h, 3 cyc/row, fp32-grade.
  bf16x3 - same 3-pass split in bf16 (11x11-bit products are exact in the
           fp32 PSUM): fp32-grade (~3e-8 d2 err). SHIPPED MODE.
The -sq_k augmented rows are always split hi/residual so they are exact
under the mode's rounding.

Measured on the 8-core axon TRN2 (vs CPU-jax reference): HW exec ~575 us,
counts exact on 9914/10000 rows (rest off by 1 from borderline fp ties;
the 0.5 threshold deliberately straddles the data), ids_selected 1992/2000
exact, aggregate rel err ~2e-4. fp32 full-matrix baseline was 1.46 ms.
"""

import numpy as np

C, P, D = 10, 1000, 1024
N = C * P  # 10000 real rows
PP = 1024  # class size padded to 1024 (M=128 tiles -> FWL; KC=512 banks)
NP = C * PP  # 10240 padded rows
N_CORES = 8
QT = 128  # query tile rows (8 tiles per padded class -> 1 per core)
NQT = C  # query tiles per core (one per class)
QPC = QT * NQT  # 1280 queries per core
KC = 512  # key chunk (matmul free dim, exactly one fp32 PSUM bank)
NKC = NP // KC  # 20 chunks; chunk c has class c//2
KS = D // 128  # 8 contraction slices
PAD_NEG = np.float32(-1.0e9)  # kaug value for padded keys: never counted
PAD_POS = np.float32(1.0e9)  # qthr value for padded queries: all-zero mask rows

MM_MODE = "bf16x3"

_PROG_CACHE = {}


def _rne(a, k):
    """Round fp32 array to (23-k) explicit mantissa bits, round-to-nearest-even."""
    u = a.view(np.uint32).astype(np.uint64)
    bias = ((u >> k) & 1) + np.uint64((1 << (k - 1)) - 1)
    r = (((u + bias) >> k) << k) & np.uint64(0xFFFFFFFF)
    return r.astype(np.uint32).view(np.float32)


def _mode_cfg(mm_mode):
    return {
        "f32": dict(split=False, n_aug=1),
        "f32r": dict(split=False, n_aug=2),
        "f32r3": dict(split=True, n_aug=2),
        "bf16x3": dict(split=True, n_aug=3),
    }[mm_mode]


def _build_program(mm_mode):
    import concourse.mybir as mybir
    from concourse import bacc
    from concourse.tile import TileContext

    f32 = mybir.dt.float32
    if mm_mode == "f32":
        mm_dt = f32
    elif mm_mode == "bf16x3":
        mm_dt = mybir.dt.bfloat16
    else:
        mm_dt = mybir.dt.float32r
    cfg = _mode_cfg(mm_mode)
    split, n_aug = cfg["split"], cfg["n_aug"]

    nc = bacc.Bacc("TRN2", target_bir_lowering=False, debug=False, num_devices=N_CORES)

    # qT holds 2*Q^T k-slices (2x folded into the stationary operand so PSUM
    # accumulates 2G directly); in split modes qT/kT are hi parts, qTl/kTl
    # the residuals.
    qT = nc.dram_tensor("qT", [128, KS, QPC], mm_dt, kind="ExternalInput").ap()
    kT = nc.dram_tensor("kT", [128, KS, NP], mm_dt, kind="ExternalInput").ap()
    if split:
        qTl = nc.dram_tensor("qTl", [128, KS, QPC], mm_dt, kind="ExternalInput").ap()
        kTl = nc.dram_tensor("kTl", [128, KS, NP], mm_dt, kind="ExternalInput").ap()
    kaug = nc.dram_tensor("kaug", [n_aug, NP], mm_dt, kind="ExternalInput").ap()
    onesw = nc.dram_tensor("onesw", [n_aug, QT], mm_dt, kind="ExternalInput").ap()
    onecol = nc.dram_tensor("onecol", [QT, 1], mm_dt, kind="ExternalInput").ap()
    qthr = nc.dram_tensor("qthr", [QT, NQT], f32, kind="ExternalInput").ap()
    cnt_out = nc.dram_tensor("cnt", [QT, NQT], f32, kind="ExternalOutput").ap()
    col_out = nc.dram_tensor("colcnt", [1, NP], f32, kind="ExternalOutput").ap()

    with TileContext(nc) as tc:
        with (
            tc.tile_pool(name="qpool", bufs=1) as qpool,
            tc.tile_pool(name="kpool", bufs=2 if split else 3) as kpool,
            tc.tile_pool(name="ktpool", bufs=3) as ktpool,
            tc.tile_pool(name="mpool", bufs=4) as mpool,
            tc.tile_pool(name="small", bufs=1) as spool,
            tc.tile_pool(name="psum", bufs=5, space="PSUM") as pspool,
            tc.tile_pool(name="psum2", bufs=2, space="PSUM") as ps2pool,
        ):
            qt_all = qpool.tile([128, KS, QPC], mm_dt, tag="qhi")
            nc.sync.dma_start(out=qt_all[:], in_=qT[:])
            # first chunk's hi operands queue ahead of the lo residuals so the
            # first matmul group is not gated on the residual transfers
            kt0 = ktpool.tile([128, KS, KC], mm_dt, tag="kt")
            nc.sync.dma_start(out=kt0[:], in_=kT[:, :, 2 * KC : 3 * KC])
            ka0 = kpool.tile([n_aug, KC], mm_dt, tag="ka")
            nc.sync.dma_start(out=ka0[:], in_=kaug[:, 2 * KC : 3 * KC])
            qtl_all = None
            if split:
                qtl_all = qpool.tile([128, KS, QPC], mm_dt, tag="qlo")
                nc.sync.dma_start(out=qtl_all[:], in_=qTl[:])

            qthr_t = spool.tile([QT, NQT], f32, tag="qthr")
            nc.sync.dma_start(out=qthr_t[:], in_=qthr[:])
            ones_t = spool.tile([n_aug, QT], mm_dt, tag="ones")
            nc.sync.dma_start(out=ones_t[:], in_=onesw[:])
            onecol_t = spool.tile([QT, 1], mm_dt, tag="onecol")
            nc.sync.dma_start(out=onecol_t[:], in_=onecol[:])

            acc = spool.tile([QT, NQT * 18], f32, tag="acc")
            nc.vector.memset(acc[:], 0.0)
            colcnt = spool.tile([1, NP], f32, tag="colcnt")
            cnt_t = spool.tile([QT, NQT], f32, tag="cnt")

            col = [0] * NQT
            for c in range(2, NKC):
                ts_list = list(range(c // 2))  # query classes strictly below
                if c == 2:
                    kt, ka = kt0, ka0
                else:
                    kt = ktpool.tile([128, KS, KC], mm_dt, tag="kt")
                    nc.sync.dma_start(
                        out=kt[:], in_=kT[:, :, c * KC : (c + 1) * KC]
                    )
                    ka = kpool.tile([n_aug, KC], mm_dt, tag="ka")
                    nc.sync.dma_start(out=ka[:], in_=kaug[:, c * KC : (c + 1) * KC])
                if split:
                    ktl = kpool.tile([128, KS, KC], mm_dt, tag="ktl")
                    nc.sync.dma_start(
                        out=ktl[:], in_=kTl[:, :, c * KC : (c + 1) * KC]
                    )

                masks = []
                for t in ts_list:
                    ps = pspool.tile([QT, KC], f32)
                    for ks in range(KS):
                        nc.tensor.matmul(
                            ps[:],
                            qt_all[:, ks, t * QT : (t + 1) * QT],
                            kt[:, ks, :],
                            start=(ks == 0),
                            stop=False,
                        )
                    if split:
                        for ks in range(KS):
                            nc.tensor.matmul(
                                ps[:],
                                qt_all[:, ks, t * QT : (t + 1) * QT],
                                ktl[:, ks, :],
                                start=False,
                                stop=False,
                            )
                        for ks in range(KS):
                            nc.tensor.matmul(
                                ps[:],
                                qtl_all[:, ks, t * QT : (t + 1) * QT],
                                kt[:, ks, :],
                                start=False,
                                stop=False,
                            )
                    nc.tensor.matmul(ps[:], ones_t[:], ka[:], start=False, stop=True)
                    mask = mpool.tile([QT, KC], mm_dt, tag="mask")
                    nc.vector.tensor_scalar(
                        out=mask[:],
                        in0=ps[:],
                        scalar1=qthr_t[:, t : t + 1],
                        scalar2=None,
                        op0=mybir.AluOpType.is_gt,
                        op1=mybir.AluOpType.add,
                        accum_out=acc[:, t * 18 + col[t] : t * 18 + col[t] + 1],
                    )
                    col[t] += 1
                    masks.append(mask)

                # column sums for this chunk's keys: ones^T @ mask, accumulated
                # over the chunk's query tiles in one PSUM group.
                ps2 = ps2pool.tile([1, KC], f32)
                for i, mask in enumerate(masks):
                    nc.tensor.matmul(
                        ps2[:],
                        onecol_t[:],
                        mask[:],
                        start=(i == 0),
                        stop=(i == len(masks) - 1),
                    )
                nc.vector.tensor_copy(colcnt[:, c * KC : (c + 1) * KC], ps2[:])

            for t in range(NQT):
                nc.vector.tensor_reduce(
                    cnt_t[:, t : t + 1],
                    acc[:, t * 18 : (t + 1) * 18],
                    axis=mybir.AxisListType.X,
                    op=mybir.AluOpType.add,
                )
            nc.sync.dma_start(out=cnt_out[:], in_=cnt_t[:])
            nc.sync.dma_start(out=col_out[:], in_=colcnt[:])

    nc.compile()
    return nc


def _get_program(mm_mode):
    if mm_mode not in _PROG_CACHE:
        _PROG_CACHE[mm_mode] = _build_program(mm_mode)
    return _PROG_CACHE[mm_mode]


def _split_hi_lo(a, mm_mode):
    if mm_mode == "bf16x3":
        import ml_dtypes

        hi = a.astype(ml_dtypes.bfloat16)
        lo = (a - hi.astype(np.float32)).astype(ml_dtypes.bfloat16)
        return np.ascontiguousarray(hi), np.ascontiguousarray(lo)
    hi = _rne(a, 13)
    return np.ascontiguousarray(hi), np.ascontiguousarray(a - hi)


def _aug_rows(msq, mm_mode):
    """Split -sq into n_aug rows whose sum is exact under the mode's rounding."""
    if mm_mode == "f32":
        return msq.reshape(1, -1).copy()
    if mm_mode == "bf16x3":
        import ml_dtypes

        h = msq.astype(ml_dtypes.bfloat16)
        r1 = msq - h.astype(np.float32)
        mid = r1.astype(ml_dtypes.bfloat16)
        lo = (r1 - mid.astype(np.float32)).astype(ml_dtypes.bfloat16)
        return np.ascontiguousarray(
            np.stack(
                [h.astype(np.float32), mid.astype(np.float32), lo.astype(np.float32)],
                axis=0,
            )
        )
    hi = _rne(msq, 13)
    return np.ascontiguousarray(np.stack([hi, msq - hi], axis=0))


def _np_dt(mm_mode):
    if mm_mode == "bf16x3":
        import ml_dtypes

        return ml_dtypes.bfloat16
    return np.float32


def _prepare_inputs(X, sq, mm_mode):
    """Build per-core in_maps from real X [N, D] f32 and sq [N] f32."""
    split = _mode_cfg(mm_mode)["split"]
    dt = _np_dt(mm_mode)

    # pad each class block from P=1000 to PP=1024 rows of zeros
    Xp = np.zeros((NP, D), dtype=np.float32)
    pad_mask = np.ones(NP, dtype=bool)  # True = padded row
    msq_p = np.full(NP, PAD_NEG, dtype=np.float32)  # -sq, PAD_NEG for pads
    thr_p = np.full(NP, PAD_POS, dtype=np.float32)  # sq-0.25, PAD_POS for pads
    for cls in range(C):
        Xp[cls * PP : cls * PP + P] = X[cls * P : (cls + 1) * P]
        pad_mask[cls * PP : cls * PP + P] = False
        msq_p[cls * PP : cls * PP + P] = -sq[cls * P : (cls + 1) * P]
        thr_p[cls * PP : cls * PP + P] = sq[cls * P : (cls + 1) * P] - np.float32(
            0.25
        )

    kT_full = np.ascontiguousarray(Xp.T.reshape(KS, 128, NP).transpose(1, 0, 2))
    if split:
        kT_hi, kT_lo = _split_hi_lo(kT_full, mm_mode)
    else:
        kT_hi = np.ascontiguousarray(kT_full.astype(dt))
        kT_lo = None

    kaug_full = np.ascontiguousarray(_aug_rows(msq_p, mm_mode).astype(dt))
    n_aug = kaug_full.shape[0]

    in_maps = []
    for core in range(N_CORES):
        rows = np.concatenate(
            [
                np.arange(cls * PP + core * QT, cls * PP + (core + 1) * QT)
                for cls in range(C)
            ]
        )
        Q2 = 2.0 * Xp[rows]  # exact scaling
        qT_c = np.ascontiguousarray(Q2.T.reshape(KS, 128, QPC).transpose(1, 0, 2))
        qthr_c = np.ascontiguousarray(thr_p[rows].reshape(NQT, QT).T)
        m = {
            "kT": kT_hi,
            "kaug": kaug_full,
            "qthr": qthr_c,
            "onesw": np.ones((n_aug, QT), dtype=dt),
            "onecol": np.ones((QT, 1), dtype=dt),
        }
        if split:
            m["qT"], m["qTl"] = _split_hi_lo(qT_c, mm_mode)
            m["kTl"] = kT_lo
        else:
            m["qT"] = np.ascontiguousarray(qT_c.astype(dt))
        in_maps.append(m)
    return in_maps


def _counts_from_results(results):
    counts_p = np.zeros(NP, dtype=np.int64)
    for core in range(N_CORES):  # row contributions (each row owned by 1 core)
        out = results[core]["cnt"]  # [QT, NQT] f32
        for cls in range(C):
            counts_p[cls * PP + core * QT : cls * PP + (core + 1) * QT] = out[
                :, cls
            ].astype(np.int64)
    for core in range(N_CORES):  # column contributions (summed over cores)
        cc = results[core]["colcnt"].reshape(NP)
        counts_p[PP:] += cc[PP:].astype(np.int64)  # class-0 keys have none
    # strip padding: real rows are the first P of each PP block
    return counts_p.reshape(C, PP)[:, :P].reshape(-1)


def kernel(feats, ids_per_cls, budget, _bench=None):
    from concourse.bass_utils import run_bass_kernel_spmd

    feats = np.asarray(feats, dtype=np.float32)
    ids_per_cls = np.asarray(ids_per_cls)
    budget_i = int(np.asarray(budget))

    ids_flat = ids_per_cls.reshape(-1).astype(np.int64)
    X = np.ascontiguousarray(feats[ids_flat])  # [N, D] class-blocked
    sq = (X.astype(np.float64) ** 2).sum(axis=1).astype(np.float32)

    nc = _get_program(MM_MODE)
    in_maps = _prepare_inputs(X, sq, MM_MODE)
    kw = dict(_bench) if _bench else {}
    res = run_bass_kernel_spmd(nc, in_maps, core_ids=list(range(N_CORES)), **kw)
    counts = _counts_from_results(res.results)

    counts = counts.reshape(C, P)
    per_cls_budget = budget_i // C
    order = np.argsort(counts, axis=-1, kind="stable")
    sel = order[:, :per_cls_budget]
    ids_selected = np.take_along_axis(
        ids_per_cls.reshape(C, P), sel, axis=1
    ).reshape(-1)

    cnt_dt = np.int64 if ids_per_cls.dtype == np.int64 else np.int32
    counts_out = counts.astype(cnt_dt)
    if _bench is not None:
        return (ids_selected, counts_out), res
    return ids_selected, counts_out


# revision 25
# speedup vs baseline: 1.0042x; 1.0033x over previous
"""Trainium2 Bass kernel for nn_CM_sampler (retrieval_knn).

Counts, for each of 10000 class-blocked representatives (10 classes x 1000),
the other-class representatives within euclidean distance 0.5 (gram trick:
d2 = sq_i + sq_j - 2*X@X.T, count d2 < 0.25), then selects per class the
budget//C lowest-count rows (host argsort, tiny).

Layout: classes are zero-padded 1000 -> 1024 rows so query tiles are
[128 x D] (full PE width, FWL weight loads) and key chunks are 512 wide
(exactly one fp32 PSUM bank). Padded keys carry a -1e9 augment (never
counted); padded queries carry a +1e9 threshold (all-zero mask rows), so
padding never perturbs real counts.

Sharding: each of the 8 cores gets one 128-row query tile from EACH class
(rows [cls*1024 + core*128, +128)), so the class-skip pattern is
core-invariant and one SPMD program serves all 8 cores. All 10240 keys are
streamed on every core in 20 chunks of 512 (chunk c has class c//2).

Symmetry: the distance matrix is symmetric, so only class pairs
(query class t) < (key class c//2) are computed. Each [128 x 512] block
yields BOTH the row contribution (fused DVE is_gt + free-axis accumulate ->
counts for this core's class-t queries vs higher classes) and the column
contribution (ones^T @ mask matmul -> counts for the chunk's keys vs this
core's class-t queries). Column contributions are summed across cores on
the host; all counts are exact small integers in fp32.

Device math per block:
  PSUM  = 2*Q @ K^T - sq_k[None, :]   (matmul passes + augmented exact rows)
  mask  = [PSUM > (sq_q - 0.25)]      (fused DVE is_gt; accum_out = row sums)
  colsum += ones^T @ mask             (PE, accumulated per chunk in PSUM)

Matmul modes (pass structure for a*b):
  f32    - plain fp32 matmuls (4 cyc/row on TensorE).
  f32r   - TF32-like single pass (1 cyc/row): the HW rounds both operands
           to ~11-bit mantissas (RNE); d2 err ~2e-6, fastest but loosest.
  f32r3  - hi/lo split (RNE-at-13, grid-aligned below the HW's ~12-bit
           rounding): h@h + h@l + l@h, 3 cyc/row, fp32-grade.
  bf16x3 - same 3-pass split in bf16 (11x11-bit products are exact in the
           fp32 PSUM): fp32-grade (~3e-8 d2 err). SHIPPED MODE.
The -sq_k augmented rows are always split hi/residual so they are exact
under the mode's rounding.

Measured on the 8-core axon TRN2 (vs CPU-jax reference): HW exec ~575 us,
counts exact on 9914/10000 rows (rest off by 1 from borderline fp ties;
the 0.5 threshold deliberately straddles the data), ids_selected 1992/2000
exact, aggregate rel err ~2e-4. fp32 full-matrix baseline was 1.46 ms.
"""

import numpy as np

C, P, D = 10, 1000, 1024
N = C * P  # 10000 real rows
PP = 1024  # class size padded to 1024 (M=128 tiles -> FWL; KC=512 banks)
NP = C * PP  # 10240 padded rows
N_CORES = 8
QT = 128  # query tile rows (8 tiles per padded class -> 1 per core)
NQT = C  # query tiles per core (one per class)
QPC = QT * NQT  # 1280 queries per core
KC = 512  # key chunk (matmul free dim, exactly one fp32 PSUM bank)
NKC = NP // KC  # 20 chunks; chunk c has class c//2
KS = D // 128  # 8 contraction slices
PAD_NEG = np.float32(-1.0e9)  # kaug value for padded keys: never counted
PAD_POS = np.float32(1.0e9)  # qthr value for padded queries: all-zero mask rows

MM_MODE = "bf16x3"

_PROG_CACHE = {}


def _rne(a, k):
    """Round fp32 array to (23-k) explicit mantissa bits, round-to-nearest-even."""
    u = a.view(np.uint32).astype(np.uint64)
    bias = ((u >> k) & 1) + np.uint64((1 << (k - 1)) - 1)
    r = (((u + bias) >> k) << k) & np.uint64(0xFFFFFFFF)
    return r.astype(np.uint32).view(np.float32)


def _mode_cfg(mm_mode):
    return {
        "f32": dict(split=False, n_aug=1),
        "f32r": dict(split=False, n_aug=2),
        "f32r3": dict(split=True, n_aug=2),
        "bf16x3": dict(split=True, n_aug=3),
    }[mm_mode]


def _build_program(mm_mode):
    import concourse.mybir as mybir
    from concourse import bacc
    from concourse.tile import TileContext

    f32 = mybir.dt.float32
    if mm_mode == "f32":
        mm_dt = f32
    elif mm_mode == "bf16x3":
        mm_dt = mybir.dt.bfloat16
    else:
        mm_dt = mybir.dt.float32r
    cfg = _mode_cfg(mm_mode)
    split, n_aug = cfg["split"], cfg["n_aug"]

    nc = bacc.Bacc("TRN2", target_bir_lowering=False, debug=False, num_devices=N_CORES)

    # qT holds 2*Q^T k-slices (2x folded into the stationary operand so PSUM
    # accumulates 2G directly); in split modes qT/kT are hi parts, qTl/kTl
    # the residuals.
    qT = nc.dram_tensor("qT", [128, KS, QPC], mm_dt, kind="ExternalInput").ap()
    kT = nc.dram_tensor("kT", [128, KS, NP], mm_dt, kind="ExternalInput").ap()
    if split:
        qTl = nc.dram_tensor("qTl", [128, KS, QPC], mm_dt, kind="ExternalInput").ap()
        kTl = nc.dram_tensor("kTl", [128, KS, NP], mm_dt, kind="ExternalInput").ap()
    kaug = nc.dram_tensor("kaug", [n_aug, NP], mm_dt, kind="ExternalInput").ap()
    onesw = nc.dram_tensor("onesw", [n_aug, QT], mm_dt, kind="ExternalInput").ap()
    onecol = nc.dram_tensor("onecol", [QT, 1], mm_dt, kind="ExternalInput").ap()
    qthr = nc.dram_tensor("qthr", [QT, NQT], f32, kind="ExternalInput").ap()
    cnt_out = nc.dram_tensor("cnt", [QT, NQT], f32, kind="ExternalOutput").ap()
    col_out = nc.dram_tensor("colcnt", [1, NP], f32, kind="ExternalOutput").ap()

    with TileContext(nc) as tc:
        with (
            tc.tile_pool(name="qpool", bufs=1) as qpool,
            tc.tile_pool(name="kpool", bufs=2 if split else 3) as kpool,
            tc.tile_pool(name="ktpool", bufs=3) as ktpool,
            tc.tile_pool(name="mpool", bufs=4) as mpool,
            tc.tile_pool(name="small", bufs=1) as spool,
            tc.tile_pool(name="psum", bufs=5, space="PSUM") as pspool,
            tc.tile_pool(name="psum2", bufs=2, space="PSUM") as ps2pool,
        ):
            qt_all = qpool.tile([128, KS, QPC], mm_dt, tag="qhi")
            nc.sync.dma_start(out=qt_all[:], in_=qT[:])
            # first chunk's hi operands queue ahead of the lo residuals so the
            # first matmul group is not gated on the residual transfers
            kt0 = ktpool.tile([128, KS, KC], mm_dt, tag="kt")
            nc.sync.dma_start(out=kt0[:], in_=kT[:, :, 2 * KC : 3 * KC])
            ka0 = kpool.tile([n_aug, KC], mm_dt, tag="ka")
            nc.sync.dma_start(out=ka0[:], in_=kaug[:, 2 * KC : 3 * KC])
            qtl_all = None
            if split:
                qtl_all = qpool.tile([128, KS, QPC], mm_dt, tag="qlo")
                nc.sync.dma_start(out=qtl_all[:], in_=qTl[:])

            qthr_t = spool.tile([QT, NQT], f32, tag="qthr")
            nc.sync.dma_start(out=qthr_t[:], in_=qthr[:])
            ones_t = spool.tile([n_aug, QT], mm_dt, tag="ones")
            nc.sync.dma_start(out=ones_t[:], in_=onesw[:])
            onecol_t = spool.tile([QT, 1], mm_dt, tag="onecol")
            nc.sync.dma_start(out=onecol_t[:], in_=onecol[:])

            acc = spool.tile([QT, NQT * 18], f32, tag="acc")
            nc.vector.memset(acc[:], 0.0)
            colcnt = spool.tile([1, NP], f32, tag="colcnt")
            cnt_t = spool.tile([QT, NQT], f32, tag="cnt")

            col = [0] * NQT
            for c in range(2, NKC):
                ts_list = list(range(c // 2))  # query classes strictly below
                if c == 2:
                    kt, ka = kt0, ka0
                else:
                    kt = ktpool.tile([128, KS, KC], mm_dt, tag="kt")
                    nc.sync.dma_start(
                        out=kt[:], in_=kT[:, :, c * KC : (c + 1) * KC]
                    )
                    ka = kpool.tile([n_aug, KC], mm_dt, tag="ka")
                    nc.sync.dma_start(out=ka[:], in_=kaug[:, c * KC : (c + 1) * KC])
                if split:
                    ktl = kpool.tile([128, KS, KC], mm_dt, tag="ktl")
                    nc.sync.dma_start(
                        out=ktl[:], in_=kTl[:, :, c * KC : (c + 1) * KC]
                    )

                masks = []
                for t in ts_list:
                    ps = pspool.tile([QT, KC], f32)
                    for ks in range(KS):
                        nc.tensor.matmul(
                            ps[:],
                            qt_all[:, ks, t * QT : (t + 1) * QT],
                            kt[:, ks, :],
                            start=(ks == 0),
                            stop=False,
                        )
                    if split:
                        for ks in range(KS):
                            nc.tensor.matmul(
                                ps[:],
                                qt_all[:, ks, t * QT : (t + 1) * QT],
                                ktl[:, ks, :],
                                start=False,
                                stop=False,
                            )
                        for ks in range(KS):
                            nc.tensor.matmul(
                                ps[:],
                                qtl_all[:, ks, t * QT : (t + 1) * QT],
                                kt[:, ks, :],
                                start=False,
                                stop=False,
                            )
                    nc.tensor.matmul(ps[:], ones_t[:], ka[:], start=False, stop=True)
                    mask = mpool.tile([QT, KC], mm_dt, tag="mask")
                    nc.vector.tensor_scalar(
                        out=mask[:],
                        in0=ps[:],
                        scalar1=qthr_t[:, t : t + 1],
                        scalar2=None,
                        op0=mybir.AluOpType.is_gt,
                        op1=mybir.AluOpType.add,
                        accum_out=acc[:, t * 18 + col[t] : t * 18 + col[t] + 1],
                    )
                    col[t] += 1
                    masks.append(mask)

                # column sums for this chunk's keys: ones^T @ mask, accumulated
                # over the chunk's query tiles in one PSUM group.
                ps2 = ps2pool.tile([1, KC], f32)
                for i, mask in enumerate(masks):
                    nc.tensor.matmul(
                        ps2[:],
                        onecol_t[:],
                        mask[:],
                        start=(i == 0),
                        stop=(i == len(masks) - 1),
                    )
                nc.vector.tensor_copy(colcnt[:, c * KC : (c + 1) * KC], ps2[:])

            for t in range(NQT):
                nc.vector.tensor_reduce(
                    cnt_t[:, t : t + 1],
                    acc[:, t * 18 : (t + 1) * 18],
                    axis=mybir.AxisListType.X,
                    op=mybir.AluOpType.add,
                )
            nc.sync.dma_start(out=cnt_out[:], in_=cnt_t[:])
            nc.sync.dma_start(out=col_out[:], in_=colcnt[:])

    nc.compile()
    return nc


def _get_program(mm_mode):
    if mm_mode not in _PROG_CACHE:
        _PROG_CACHE[mm_mode] = _build_program(mm_mode)
    return _PROG_CACHE[mm_mode]


def _split_hi_lo(a, mm_mode):
    if mm_mode == "bf16x3":
        import ml_dtypes

        hi = a.astype(ml_dtypes.bfloat16)
        lo = (a - hi.astype(np.float32)).astype(ml_dtypes.bfloat16)
        return np.ascontiguousarray(hi), np.ascontiguousarray(lo)
    hi = _rne(a, 13)
    return np.ascontiguousarray(hi), np.ascontiguousarray(a - hi)


def _aug_rows(msq, mm_mode):
    """Split -sq into n_aug rows whose sum is exact under the mode's rounding."""
    if mm_mode == "f32":
        return msq.reshape(1, -1).copy()
    if mm_mode == "bf16x3":
        import ml_dtypes

        h = msq.astype(ml_dtypes.bfloat16)
        r1 = msq - h.astype(np.float32)
        mid = r1.astype(ml_dtypes.bfloat16)
        lo = (r1 - mid.astype(np.float32)).astype(ml_dtypes.bfloat16)
        return np.ascontiguousarray(
            np.stack(
                [h.astype(np.float32), mid.astype(np.float32), lo.astype(np.float32)],
                axis=0,
            )
        )
    hi = _rne(msq, 13)
    return np.ascontiguousarray(np.stack([hi, msq - hi], axis=0))


def _np_dt(mm_mode):
    if mm_mode == "bf16x3":
        import ml_dtypes

        return ml_dtypes.bfloat16
    return np.float32


def _prepare_inputs(X, sq, mm_mode):
    """Build per-core in_maps from real X [N, D] f32 and sq [N] f32."""
    split = _mode_cfg(mm_mode)["split"]
    dt = _np_dt(mm_mode)

    # pad each class block from P=1000 to PP=1024 rows of zeros
    Xp = np.zeros((NP, D), dtype=np.float32)
    pad_mask = np.ones(NP, dtype=bool)  # True = padded row
    msq_p = np.full(NP, PAD_NEG, dtype=np.float32)  # -sq, PAD_NEG for pads
    thr_p = np.full(NP, PAD_POS, dtype=np.float32)  # sq-0.25, PAD_POS for pads
    for cls in range(C):
        Xp[cls * PP : cls * PP + P] = X[cls * P : (cls + 1) * P]
        pad_mask[cls * PP : cls * PP + P] = False
        msq_p[cls * PP : cls * PP + P] = -sq[cls * P : (cls + 1) * P]
        thr_p[cls * PP : cls * PP + P] = sq[cls * P : (cls + 1) * P] - np.float32(
            0.25
        )

    kT_full = np.ascontiguousarray(Xp.T.reshape(KS, 128, NP).transpose(1, 0, 2))
    if split:
        kT_hi, kT_lo = _split_hi_lo(kT_full, mm_mode)
    else:
        kT_hi = np.ascontiguousarray(kT_full.astype(dt))
        kT_lo = None

    kaug_full = np.ascontiguousarray(_aug_rows(msq_p, mm_mode).astype(dt))
    n_aug = kaug_full.shape[0]

    in_maps = []
    for core in range(N_CORES):
        rows = np.concatenate(
            [
                np.arange(cls * PP + core * QT, cls * PP + (core + 1) * QT)
                for cls in range(C)
            ]
        )
        Q2 = 2.0 * Xp[rows]  # exact scaling
        qT_c = np.ascontiguousarray(Q2.T.reshape(KS, 128, QPC).transpose(1, 0, 2))
        qthr_c = np.ascontiguousarray(thr_p[rows].reshape(NQT, QT).T)
        m = {
            "kT": kT_hi,
            "kaug": kaug_full,
            "qthr": qthr_c,
            "onesw": np.ones((n_aug, QT), dtype=dt),
            "onecol": np.ones((QT, 1), dtype=dt),
        }
        if split:
            m["qT"], m["qTl"] = _split_hi_lo(qT_c, mm_mode)
            m["kTl"] = kT_lo
        else:
            m["qT"] = np.ascontiguousarray(qT_c.astype(dt))
        in_maps.append(m)
    return in_maps


def _counts_from_results(results):
    counts_p = np.zeros(NP, dtype=np.int64)
    for core in range(N_CORES):  # row contributions (each row owned by 1 core)
        out = results[core]["cnt"]  # [QT, NQT] f32
        for cls in range(C):
            counts_p[cls * PP + core * QT : cls * PP + (core + 1) * QT] = out[
                :, cls
            ].astype(np.int64)
    for core in range(N_CORES):  # column contributions (summed over cores)
        cc = results[core]["colcnt"].reshape(NP)
        counts_p[PP:] += cc[PP:].astype(np.int64)  # class-0 keys have none
    # strip padding: real rows are the first P of each PP block
    return counts_p.reshape(C, PP)[:, :P].reshape(-1)


def kernel(feats, ids_per_cls, budget, _bench=None):
    from concourse.bass_utils import run_bass_kernel_spmd

    feats = np.asarray(feats, dtype=np.float32)
    ids_per_cls = np.asarray(ids_per_cls)
    budget_i = int(np.asarray(budget))

    ids_flat = ids_per_cls.reshape(-1).astype(np.int64)
    X = np.ascontiguousarray(feats[ids_flat])  # [N, D] class-blocked
    sq = (X.astype(np.float64) ** 2).sum(axis=1).astype(np.float32)

    nc = _get_program(MM_MODE)
    in_maps = _prepare_inputs(X, sq, MM_MODE)
    kw = dict(_bench) if _bench else {}
    res = run_bass_kernel_spmd(nc, in_maps, core_ids=list(range(N_CORES)), **kw)
    counts = _counts_from_results(res.results)

    counts = counts.reshape(C, P)
    per_cls_budget = budget_i // C
    order = np.argsort(counts, axis=-1, kind="stable")
    sel = order[:, :per_cls_budget]
    ids_selected = np.take_along_axis(
        ids_per_cls.reshape(C, P), sel, axis=1
    ).reshape(-1)

    cnt_dt = np.int64 if ids_per_cls.dtype == np.int64 else np.int32
    counts_out = counts.astype(cnt_dt)
    if _bench is not None:
        return (ids_selected, counts_out), res
    return ids_selected, counts_out


# revision 26
# speedup vs baseline: 1.0061x; 1.0019x over previous
"""Trainium2 Bass kernel for nn_CM_sampler (retrieval_knn).

Counts, for each of 10000 class-blocked representatives (10 classes x 1000),
the other-class representatives within euclidean distance 0.5 (gram trick:
d2 = sq_i + sq_j - 2*X@X.T, count d2 < 0.25), then selects per class the
budget//C lowest-count rows (host argsort, tiny).

Layout: classes are zero-padded 1000 -> 1024 rows so query tiles are
[128 x D] (full PE width, FWL weight loads) and key chunks are 512 wide
(exactly one fp32 PSUM bank). Padded keys carry a -1e9 augment (never
counted); padded queries carry a +1e9 threshold (all-zero mask rows), so
padding never perturbs real counts.

Sharding: each of the 8 cores gets one 128-row query tile from EACH class
(rows [cls*1024 + core*128, +128)), so the class-skip pattern is
core-invariant and one SPMD program serves all 8 cores. All 10240 keys are
streamed on every core in 20 chunks of 512 (chunk c has class c//2).

Symmetry: the distance matrix is symmetric, so only class pairs
(query class t) < (key class c//2) are computed. Each [128 x 512] block
yields BOTH the row contribution (fused DVE is_gt + free-axis accumulate ->
counts for this core's class-t queries vs higher classes) and the column
contribution (ones^T @ mask matmul -> counts for the chunk's keys vs this
core's class-t queries). Column contributions are summed across cores on
the host; all counts are exact small integers in fp32.

Device math per block:
  PSUM  = 2*Q @ K^T - sq_k[None, :]   (matmul passes + augmented exact rows)
  mask  = [PSUM > (sq_q - 0.25)]      (fused DVE is_gt; accum_out = row sums)
  colsum += ones^T @ mask             (PE, accumulated per chunk in PSUM)

Matmul modes (pass structure for a*b):
  f32    - plain fp32 matmuls (4 cyc/row on TensorE).
  f32r   - TF32-like single pass (1 cyc/row): the HW rounds both operands
           to ~11-bit mantissas (RNE); d2 err ~2e-6, fastest but loosest.
  f32r3  - hi/lo split (RNE-at-13, grid-aligned below the HW's ~12-bit
           rounding): h@h + h@l + l@h, 3 cyc/row, fp32-grade.
  bf16x3 - same 3-pass split in bf16 (11x11-bit products are exact in the
           fp32 PSUM): fp32-grade (~3e-8 d2 err). SHIPPED MODE.
The -sq_k augmented rows are always split hi/residual so they are exact
under the mode's rounding.

Measured on the 8-core axon TRN2 (vs CPU-jax reference): HW exec ~575 us,
counts exact on 9914/10000 rows (rest off by 1 from borderline fp ties;
the 0.5 threshold deliberately straddles the data), ids_selected 1992/2000
exact, aggregate rel err ~2e-4. fp32 full-matrix baseline was 1.46 ms.
"""

import numpy as np

C, P, D = 10, 1000, 1024
N = C * P  # 10000 real rows
PP = 1024  # class size padded to 1024 (M=128 tiles -> FWL; KC=512 banks)
NP = C * PP  # 10240 padded rows
N_CORES = 8
QT = 128  # query tile rows (8 tiles per padded class -> 1 per core)
NQT = C  # query tiles per core (one per class)
QPC = QT * NQT  # 1280 queries per core
KC = 512  # key chunk (matmul free dim, exactly one fp32 PSUM bank)
NKC = NP // KC  # 20 chunks; chunk c has class c//2
KS = D // 128  # 8 contraction slices
PAD_NEG = np.float32(-1.0e9)  # kaug value for padded keys: never counted
PAD_POS = np.float32(1.0e9)  # qthr value for padded queries: all-zero mask rows

MM_MODE = "bf16x3"

_PROG_CACHE = {}


def _rne(a, k):
    """Round fp32 array to (23-k) explicit mantissa bits, round-to-nearest-even."""
    u = a.view(np.uint32).astype(np.uint64)
    bias = ((u >> k) & 1) + np.uint64((1 << (k - 1)) - 1)
    r = (((u + bias) >> k) << k) & np.uint64(0xFFFFFFFF)
    return r.astype(np.uint32).view(np.float32)


def _mode_cfg(mm_mode):
    return {
        "f32": dict(split=False, n_aug=1),
        "f32r": dict(split=False, n_aug=2),
        "f32r3": dict(split=True, n_aug=2),
        "bf16x3": dict(split=True, n_aug=3),
    }[mm_mode]


def _build_program(mm_mode):
    import concourse.mybir as mybir
    from concourse import bacc
    from concourse.tile import TileContext

    f32 = mybir.dt.float32
    if mm_mode == "f32":
        mm_dt = f32
    elif mm_mode == "bf16x3":
        mm_dt = mybir.dt.bfloat16
    else:
        mm_dt = mybir.dt.float32r
    cfg = _mode_cfg(mm_mode)
    split, n_aug = cfg["split"], cfg["n_aug"]

    nc = bacc.Bacc("TRN2", target_bir_lowering=False, debug=False, num_devices=N_CORES)

    # qT holds 2*Q^T k-slices (2x folded into the stationary operand so PSUM
    # accumulates 2G directly); in split modes qT/kT are hi parts, qTl/kTl
    # the residuals.
    qT = nc.dram_tensor("qT", [128, KS, QPC], mm_dt, kind="ExternalInput").ap()
    kT = nc.dram_tensor("kT", [128, KS, NP], mm_dt, kind="ExternalInput").ap()
    if split:
        qTl = nc.dram_tensor("qTl", [128, KS, QPC], mm_dt, kind="ExternalInput").ap()
        kTl = nc.dram_tensor("kTl", [128, KS, NP], mm_dt, kind="ExternalInput").ap()
    kaug = nc.dram_tensor("kaug", [n_aug, NP], mm_dt, kind="ExternalInput").ap()
    onesw = nc.dram_tensor("onesw", [n_aug, QT], mm_dt, kind="ExternalInput").ap()
    onecol = nc.dram_tensor("onecol", [QT, 1], mm_dt, kind="ExternalInput").ap()
    qthr = nc.dram_tensor("qthr", [QT, NQT], f32, kind="ExternalInput").ap()
    cnt_out = nc.dram_tensor("cnt", [QT, NQT], f32, kind="ExternalOutput").ap()
    col_out = nc.dram_tensor("colcnt", [1, NP], f32, kind="ExternalOutput").ap()

    with TileContext(nc) as tc:
        with (
            tc.tile_pool(name="qpool", bufs=1) as qpool,
            tc.tile_pool(name="kpool", bufs=2 if split else 3) as kpool,
            tc.tile_pool(name="ktpool", bufs=3) as ktpool,
            tc.tile_pool(name="mpool", bufs=4) as mpool,
            tc.tile_pool(name="small", bufs=1) as spool,
            tc.tile_pool(name="psum", bufs=5, space="PSUM") as pspool,
            tc.tile_pool(name="psum2", bufs=2, space="PSUM") as ps2pool,
        ):
            qt_all = qpool.tile([128, KS, QPC], mm_dt, tag="qhi")
            nc.sync.dma_start(out=qt_all[:], in_=qT[:])
            # first chunk's hi operands queue ahead of the lo residuals so the
            # first matmul group is not gated on the residual transfers
            kt0 = ktpool.tile([128, KS, KC], mm_dt, tag="kt")
            nc.sync.dma_start(out=kt0[:], in_=kT[:, :, 2 * KC : 3 * KC])
            ka0 = kpool.tile([n_aug, KC], mm_dt, tag="ka")
            nc.sync.dma_start(out=ka0[:], in_=kaug[:, 2 * KC : 3 * KC])
            qtl_all = None
            if split:
                qtl_all = qpool.tile([128, KS, QPC], mm_dt, tag="qlo")
                # residual stream rides the ScalarE HWDGE ring, in parallel
                # with the hi stream on the sync ring
                nc.scalar.dma_start(out=qtl_all[:], in_=qTl[:])

            qthr_t = spool.tile([QT, NQT], f32, tag="qthr")
            nc.sync.dma_start(out=qthr_t[:], in_=qthr[:])
            ones_t = spool.tile([n_aug, QT], mm_dt, tag="ones")
            nc.sync.dma_start(out=ones_t[:], in_=onesw[:])
            onecol_t = spool.tile([QT, 1], mm_dt, tag="onecol")
            nc.sync.dma_start(out=onecol_t[:], in_=onecol[:])

            acc = spool.tile([QT, NQT * 18], f32, tag="acc")
            nc.vector.memset(acc[:], 0.0)
            colcnt = spool.tile([1, NP], f32, tag="colcnt")
            cnt_t = spool.tile([QT, NQT], f32, tag="cnt")

            col = [0] * NQT
            for c in range(2, NKC):
                ts_list = list(range(c // 2))  # query classes strictly below
                if c == 2:
                    kt, ka = kt0, ka0
                else:
                    kt = ktpool.tile([128, KS, KC], mm_dt, tag="kt")
                    nc.sync.dma_start(
                        out=kt[:], in_=kT[:, :, c * KC : (c + 1) * KC]
                    )
                    ka = kpool.tile([n_aug, KC], mm_dt, tag="ka")
                    nc.sync.dma_start(out=ka[:], in_=kaug[:, c * KC : (c + 1) * KC])
                if split:
                    ktl = kpool.tile([128, KS, KC], mm_dt, tag="ktl")
                    nc.scalar.dma_start(
                        out=ktl[:], in_=kTl[:, :, c * KC : (c + 1) * KC]
                    )

                masks = []
                for t in ts_list:
                    ps = pspool.tile([QT, KC], f32)
                    for ks in range(KS):
                        nc.tensor.matmul(
                            ps[:],
                            qt_all[:, ks, t * QT : (t + 1) * QT],
                            kt[:, ks, :],
                            start=(ks == 0),
                            stop=False,
                        )
                    if split:
                        for ks in range(KS):
                            nc.tensor.matmul(
                                ps[:],
                                qt_all[:, ks, t * QT : (t + 1) * QT],
                                ktl[:, ks, :],
                                start=False,
                                stop=False,
                            )
                        for ks in range(KS):
                            nc.tensor.matmul(
                                ps[:],
                                qtl_all[:, ks, t * QT : (t + 1) * QT],
                                kt[:, ks, :],
                                start=False,
                                stop=False,
                            )
                    nc.tensor.matmul(ps[:], ones_t[:], ka[:], start=False, stop=True)
                    mask = mpool.tile([QT, KC], mm_dt, tag="mask")
                    nc.vector.tensor_scalar(
                        out=mask[:],
                        in0=ps[:],
                        scalar1=qthr_t[:, t : t + 1],
                        scalar2=None,
                        op0=mybir.AluOpType.is_gt,
                        op1=mybir.AluOpType.add,
                        accum_out=acc[:, t * 18 + col[t] : t * 18 + col[t] + 1],
                    )
                    col[t] += 1
                    masks.append(mask)

                # column sums for this chunk's keys: ones^T @ mask, accumulated
                # over the chunk's query tiles in one PSUM group.
                ps2 = ps2pool.tile([1, KC], f32)
                for i, mask in enumerate(masks):
                    nc.tensor.matmul(
                        ps2[:],
                        onecol_t[:],
                        mask[:],
                        start=(i == 0),
                        stop=(i == len(masks) - 1),
                    )
                nc.vector.tensor_copy(colcnt[:, c * KC : (c + 1) * KC], ps2[:])

            for t in range(NQT):
                nc.vector.tensor_reduce(
                    cnt_t[:, t : t + 1],
                    acc[:, t * 18 : (t + 1) * 18],
                    axis=mybir.AxisListType.X,
                    op=mybir.AluOpType.add,
                )
            nc.sync.dma_start(out=cnt_out[:], in_=cnt_t[:])
            nc.sync.dma_start(out=col_out[:], in_=colcnt[:])

    nc.compile()
    return nc


def _get_program(mm_mode):
    if mm_mode not in _PROG_CACHE:
        _PROG_CACHE[mm_mode] = _build_program(mm_mode)
    return _PROG_CACHE[mm_mode]


def _split_hi_lo(a, mm_mode):
    if mm_mode == "bf16x3":
        import ml_dtypes

        hi = a.astype(ml_dtypes.bfloat16)
        lo = (a - hi.astype(np.float32)).astype(ml_dtypes.bfloat16)
        return np.ascontiguousarray(hi), np.ascontiguousarray(lo)
    hi = _rne(a, 13)
    return np.ascontiguousarray(hi), np.ascontiguousarray(a - hi)


def _aug_rows(msq, mm_mode):
    """Split -sq into n_aug rows whose sum is exact under the mode's rounding."""
    if mm_mode == "f32":
        return msq.reshape(1, -1).copy()
    if mm_mode == "bf16x3":
        import ml_dtypes

        h = msq.astype(ml_dtypes.bfloat16)
        r1 = msq - h.astype(np.float32)
        mid = r1.astype(ml_dtypes.bfloat16)
        lo = (r1 - mid.astype(np.float32)).astype(ml_dtypes.bfloat16)
        return np.ascontiguousarray(
            np.stack(
                [h.astype(np.float32), mid.astype(np.float32), lo.astype(np.float32)],
                axis=0,
            )
        )
    hi = _rne(msq, 13)
    return np.ascontiguousarray(np.stack([hi, msq - hi], axis=0))


def _np_dt(mm_mode):
    if mm_mode == "bf16x3":
        import ml_dtypes

        return ml_dtypes.bfloat16
    return np.float32


def _prepare_inputs(X, sq, mm_mode):
    """Build per-core in_maps from real X [N, D] f32 and sq [N] f32."""
    split = _mode_cfg(mm_mode)["split"]
    dt = _np_dt(mm_mode)

    # pad each class block from P=1000 to PP=1024 rows of zeros
    Xp = np.zeros((NP, D), dtype=np.float32)
    pad_mask = np.ones(NP, dtype=bool)  # True = padded row
    msq_p = np.full(NP, PAD_NEG, dtype=np.float32)  # -sq, PAD_NEG for pads
    thr_p = np.full(NP, PAD_POS, dtype=np.float32)  # sq-0.25, PAD_POS for pads
    for cls in range(C):
        Xp[cls * PP : cls * PP + P] = X[cls * P : (cls + 1) * P]
        pad_mask[cls * PP : cls * PP + P] = False
        msq_p[cls * PP : cls * PP + P] = -sq[cls * P : (cls + 1) * P]
        thr_p[cls * PP : cls * PP + P] = sq[cls * P : (cls + 1) * P] - np.float32(
            0.25
        )

    kT_full = np.ascontiguousarray(Xp.T.reshape(KS, 128, NP).transpose(1, 0, 2))
    if split:
        kT_hi, kT_lo = _split_hi_lo(kT_full, mm_mode)
    else:
        kT_hi = np.ascontiguousarray(kT_full.astype(dt))
        kT_lo = None

    kaug_full = np.ascontiguousarray(_aug_rows(msq_p, mm_mode).astype(dt))
    n_aug = kaug_full.shape[0]

    in_maps = []
    for core in range(N_CORES):
        rows = np.concatenate(
            [
                np.arange(cls * PP + core * QT, cls * PP + (core + 1) * QT)
                for cls in range(C)
            ]
        )
        Q2 = 2.0 * Xp[rows]  # exact scaling
        qT_c = np.ascontiguousarray(Q2.T.reshape(KS, 128, QPC).transpose(1, 0, 2))
        qthr_c = np.ascontiguousarray(thr_p[rows].reshape(NQT, QT).T)
        m = {
            "kT": kT_hi,
            "kaug": kaug_full,
            "qthr": qthr_c,
            "onesw": np.ones((n_aug, QT), dtype=dt),
            "onecol": np.ones((QT, 1), dtype=dt),
        }
        if split:
            m["qT"], m["qTl"] = _split_hi_lo(qT_c, mm_mode)
            m["kTl"] = kT_lo
        else:
            m["qT"] = np.ascontiguousarray(qT_c.astype(dt))
        in_maps.append(m)
    return in_maps


def _counts_from_results(results):
    counts_p = np.zeros(NP, dtype=np.int64)
    for core in range(N_CORES):  # row contributions (each row owned by 1 core)
        out = results[core]["cnt"]  # [QT, NQT] f32
        for cls in range(C):
            counts_p[cls * PP + core * QT : cls * PP + (core + 1) * QT] = out[
                :, cls
            ].astype(np.int64)
    for core in range(N_CORES):  # column contributions (summed over cores)
        cc = results[core]["colcnt"].reshape(NP)
        counts_p[PP:] += cc[PP:].astype(np.int64)  # class-0 keys have none
    # strip padding: real rows are the first P of each PP block
    return counts_p.reshape(C, PP)[:, :P].reshape(-1)


def kernel(feats, ids_per_cls, budget, _bench=None):
    from concourse.bass_utils import run_bass_kernel_spmd

    feats = np.asarray(feats, dtype=np.float32)
    ids_per_cls = np.asarray(ids_per_cls)
    budget_i = int(np.asarray(budget))

    ids_flat = ids_per_cls.reshape(-1).astype(np.int64)
    X = np.ascontiguousarray(feats[ids_flat])  # [N, D] class-blocked
    sq = (X.astype(np.float64) ** 2).sum(axis=1).astype(np.float32)

    nc = _get_program(MM_MODE)
    in_maps = _prepare_inputs(X, sq, MM_MODE)
    kw = dict(_bench) if _bench else {}
    res = run_bass_kernel_spmd(nc, in_maps, core_ids=list(range(N_CORES)), **kw)
    counts = _counts_from_results(res.results)

    counts = counts.reshape(C, P)
    per_cls_budget = budget_i // C
    order = np.argsort(counts, axis=-1, kind="stable")
    sel = order[:, :per_cls_budget]
    ids_selected = np.take_along_axis(
        ids_per_cls.reshape(C, P), sel, axis=1
    ).reshape(-1)

    cnt_dt = np.int64 if ids_per_cls.dtype == np.int64 else np.int32
    counts_out = counts.astype(cnt_dt)
    if _bench is not None:
        return (ids_selected, counts_out), res
    return ids_selected, counts_out
